# revision 25
# baseline (speedup 1.0000x reference)
"""Trainium2 Bass kernel for 2-layer GAT (N=100000, E=1600000, 64-dim) + MLP.

Strategy (dst-sharded, 8 cores):
- Host: permute nodes by in-degree into 128-node blocks; slot j across cores
  shares a compile-time max-degree Kb. Per-core gather index grids [128, Kb]
  (partition = dst node, k = in-edge slot), padded with a table pad-row whose
  attention logit is -1e9 (=> exp ~ 0).
- Device per layer: build feature table slice [xh(64)|al_hi|al_lo] bf16 via
  one matmul with host-packed extended weights (al = xh@att_l folded into the
  weight matrix; ar kept per-node on-chip). AllGather table slices across the
  8 cores. Per block: indirect-DMA gather of in-edge rows, attention weights
  via ACT (lrelu+exp, segment-max skipped - logits are O(10)), weighted sum
  via DVE mult + reduce, normalize + relu. Layer-1 table built from layer-0
  output resident in SBUF; post-MLP (two linears folded into one matmul on
  host) fused into the layer-1 epilogue.
- Host: concat per-core output slices, inverse-permute.
"""
import sys

for _p in ('/opt/trn_rl_repo', '/root/.axon_site/_ro/trn_rl_repo'):
    if _p not in sys.path:
        sys.path.insert(0, _p)

import numpy as np
import ml_dtypes

import concourse.bass as bass
import concourse.bacc as bacc
import concourse.mybir as mybir
import concourse.tile as tile
from concourse.bass_utils import run_bass_kernel_spmd
from concourse.masks import make_identity

BF16 = ml_dtypes.bfloat16
P = 128
NCORES = 8
N = 100000
E = 1600000
IN_DIM = 128
C = 64           # hidden dim
CT = 66          # table cols: 64 features + al_hi + al_lo
NEG_SLOPE = 0.2

NPAD = ((N + NCORES * P - 1) // (NCORES * P)) * (NCORES * P)   # 100352
N_BLOCKS = NPAD // P                                           # 784
N_SLOTS = N_BLOCKS // NCORES                                   # 98
SLICE_NODES = N_SLOTS * P                                      # 12544
SLICE_H = SLICE_NODES + 1                                      # + pad row
PAD_ROW = SLICE_NODES                                          # global table row
TBL_H = SLICE_H * NCORES


def _preprocess(edge_index):
    src = edge_index[0].astype(np.int64)
    dst = edge_index[1].astype(np.int64)
    deg = np.bincount(dst, minlength=N)
    deg_pad = np.concatenate([deg, np.zeros(NPAD - N, np.int64)])
    perm = np.argsort(-deg_pad, kind="stable")          # perm[new_pos] = node
    # block g (sorted order) -> max degree
    blk_max = deg_pad[perm].reshape(N_BLOCKS, P).max(axis=1)
    slot_kb = np.maximum(
        blk_max.reshape(N_SLOTS, NCORES).max(axis=1), 1).astype(np.int64)
    # core i gets sorted blocks (8j + i); node order = core-major
    block_order = np.concatenate([np.arange(N_SLOTS) * NCORES + i for i in range(NCORES)])
    node_order = perm.reshape(N_BLOCKS, P)[block_order].reshape(-1)
    inv_node_order = np.empty(NPAD, np.int64)
    inv_node_order[node_order] = np.arange(NPAD)

    # CSR of in-edges per node
    order = np.argsort(dst, kind="stable")
    src_sorted = src[order]
    starts = np.zeros(N + 1, np.int64)
    np.cumsum(deg, out=starts[1:])

    # table row of node m (in AG layout)
    q = inv_node_order  # permuted position of node m
    tbl_row = (q // SLICE_NODES) * SLICE_H + (q % SLICE_NODES)   # [NPAD]

    sumk = int(slot_kb.sum())
    idx = np.full((NCORES, P, sumk), PAD_ROW, np.int32)
    offs = np.zeros(N_SLOTS + 1, np.int64)
    np.cumsum(slot_kb, out=offs[1:])
    for i in range(NCORES):
        for j in range(N_SLOTS):
            o = offs[j]
            for p in range(P):
                m = node_order[i * SLICE_NODES + j * P + p]
                if m < N:
                    s, e = starts[m], starts[m + 1]
                    rows = tbl_row[src_sorted[s:e]]
                    idx[i, p, o:o + (e - s)] = rows
    return node_order, inv_node_order, slot_kb, offs, idx


def _build_program(slot_kb, offs, sumk):
    nc = bacc.Bacc("TRN2", target_bir_lowering=False, debug=False,
                   num_devices=NCORES)
    xT = nc.dram_tensor("xT", [IN_DIM, SLICE_NODES], mybir.dt.bfloat16,
                        kind="ExternalInput")
    idx_in = nc.dram_tensor("idx_in", [P, sumk], mybir.dt.int32,
                            kind="ExternalInput")
    w0 = nc.dram_tensor("w0", [IN_DIM, CT], mybir.dt.bfloat16, kind="ExternalInput")
    w1 = nc.dram_tensor("w1", [C, CT], mybir.dt.bfloat16, kind="ExternalInput")
    wp = nc.dram_tensor("wp", [C, C], mybir.dt.float32, kind="ExternalInput")
    out_d = nc.dram_tensor("out_d", [SLICE_NODES, C], mybir.dt.float32,
                           kind="ExternalOutput")

    with tile.TileContext(nc) as tc:
        with (
            tc.tile_pool(name="dram", bufs=1, space="DRAM") as dram,
            tc.tile_pool(name="const", bufs=1) as cpool,
            tc.tile_pool(name="persist", bufs=1) as ppool,
            tc.tile_pool(name="work", bufs=6) as wpool,
            tc.tile_pool(name="psA", bufs=2, space="PSUM") as psA,
            tc.tile_pool(name="psB", bufs=2, space="PSUM") as psB,
        ):
            ident = cpool.tile([P, P], mybir.dt.float32)
            make_identity(nc, ident)
            w0_sb = cpool.tile([IN_DIM, CT], mybir.dt.bfloat16)
            nc.sync.dma_start(w0_sb[:], w0[:])
            w1_sb = cpool.tile([C, CT], mybir.dt.bfloat16)
            nc.sync.dma_start(w1_sb[:], w1[:])
            wp_sb = cpool.tile([C, C], mybir.dt.float32)
            nc.sync.dma_start(wp_sb[:], wp[:])
            idx_sb = ppool.tile([P, sumk], mybir.dt.int32)
            nc.sync.dma_start(idx_sb[:], idx_in[:])

            # bulk-load x^T once (4 large DMAs instead of 98 small ones)
            xT_sb = ppool.tile([IN_DIM, SLICE_NODES], mybir.dt.bfloat16)
            QTR = SLICE_NODES // 4
            for q in range(4):
                nc.sync.dma_start(xT_sb[:, q * QTR:(q + 1) * QTR],
                                  xT[:, q * QTR:(q + 1) * QTR])

            h0_sb = ppool.tile([P, N_SLOTS * C], mybir.dt.float32)
            ar_sb = ppool.tile([P, 2 * N_SLOTS], mybir.dt.float32)

            tabs = []
            for layer in range(2):
                tab_slice = dram.tile([SLICE_H, CT], mybir.dt.bfloat16,
                                      name=f"tab_slice{layer}")
                tab_full = dram.tile([TBL_H, CT], mybir.dt.bfloat16,
                                     addr_space="Shared", name=f"tab_full{layer}")
                tabs.append((tab_slice, tab_full))

            padrow = cpool.tile([1, CT], mybir.dt.bfloat16)
            nc.vector.memset(padrow[:], 0)
            nc.vector.memset(padrow[0:1, C:C + 1], -1e9)

            def build_table(layer):
                tab_slice, tab_full = tabs[layer]
                for b in range(N_SLOTS):
                    ps = psA.tile([P, CT], mybir.dt.float32, tag="pst")
                    if layer == 0:
                        nc.tensor.matmul(out=ps[:],
                                         lhsT=xT_sb[:, b * P:(b + 1) * P],
                                         rhs=w0_sb[:], start=True, stop=True)
                    else:
                        tp = psB.tile([C, P], mybir.dt.float32, tag="tp")
                        nc.tensor.transpose(
                            out=tp[:], in_=h0_sb[:, b * C:(b + 1) * C],
                            identity=ident[:])
                        lhs = wpool.tile([C, P], mybir.dt.bfloat16, tag="hT")
                        nc.vector.tensor_copy(out=lhs[:], in_=tp[:])
                        nc.tensor.matmul(out=ps[:], lhsT=lhs[:], rhs=w1_sb[:],
                                         start=True, stop=True)
                    tt = wpool.tile([P, CT], mybir.dt.bfloat16, tag="tt")
                    nc.vector.tensor_copy(out=tt[:, 0:C + 1], in_=ps[:, 0:C + 1])
                    # al_lo = al - bf16(al)
                    nc.vector.tensor_tensor(
                        out=tt[:, C + 1:C + 2], in0=ps[:, C:C + 1],
                        in1=tt[:, C:C + 1], op=mybir.AluOpType.subtract)
                    # keep ar
                    nc.scalar.copy(
                        out=ar_sb[:, layer * N_SLOTS + b:layer * N_SLOTS + b + 1],
                        in_=ps[:, C + 1:C + 2])
                    # layer 0: split table writes between the sync HWDGE ring
                    # and the (idle) gpsimd SWDGE path to halve the write
                    # chain; layer 1 builds while gpsimd streams gathers, so
                    # keep its writes off gpsimd
                    if layer == 0 and b % 2 == 1:
                        nc.gpsimd.dma_start(tab_slice[b * P:(b + 1) * P, :],
                                            tt[:])
                    else:
                        nc.sync.dma_start(tab_slice[b * P:(b + 1) * P, :],
                                          tt[:])
                nc.sync.dma_start(tab_slice[PAD_ROW:PAD_ROW + 1, :], padrow[:])
                nc.gpsimd.collective_compute(
                    "AllGather", mybir.AluOpType.bypass,
                    replica_groups=[list(range(NCORES))],
                    ins=[tab_slice[:]], outs=[tab_full[:]],
                )

            def edge_phase(layer):
                _, tab_full = tabs[layer]
                for b in range(N_SLOTS):
                    kb = int(slot_kb[b])
                    o = int(offs[b])
                    g = wpool.tile([P, kb * CT], mybir.dt.bfloat16, tag="g",
                                   padded_shape=[P, int(slot_kb.max()) * CT])
                    for k in range(kb):
                        nc.gpsimd.indirect_dma_start(
                            out=g[:, k * CT:(k + 1) * CT],
                            out_offset=None,
                            in_=tab_full[:],
                            in_offset=bass.IndirectOffsetOnAxis(
                                ap=idx_sb[:, o + k:o + k + 1], axis=0),
                        )
                    g3 = g[:].rearrange("p (k c) -> p k c", c=CT)
                    a_t = wpool.tile([P, kb], mybir.dt.float32, tag="a",
                                     padded_shape=[P, int(slot_kb.max())])
                    nc.vector.tensor_tensor(out=a_t[:], in0=g3[:, :, C],
                                            in1=g3[:, :, C + 1],
                                            op=mybir.AluOpType.add)
                    af = wpool.tile([P, kb], mybir.dt.float32, tag="af",
                                    padded_shape=[P, int(slot_kb.max())])
                    nc.vector.tensor_scalar(
                        out=af[:], in0=a_t[:],
                        scalar1=ar_sb[:, layer * N_SLOTS + b:layer * N_SLOTS + b + 1],
                        scalar2=None, op0=mybir.AluOpType.add)
                    am = wpool.tile([P, kb], mybir.dt.float32, tag="am",
                                    padded_shape=[P, int(slot_kb.max())])
                    nc.vector.tensor_scalar(
                        out=am[:], in0=af[:], scalar1=NEG_SLOPE, scalar2=None,
                        op0=mybir.AluOpType.mult)
                    a2 = wpool.tile([P, kb], mybir.dt.float32, tag="a2",
                                    padded_shape=[P, int(slot_kb.max())])
                    nc.vector.tensor_tensor(out=a2[:], in0=af[:], in1=am[:],
                                            op=mybir.AluOpType.max)
                    ev = wpool.tile([P, kb], mybir.dt.bfloat16, tag="ev",
                                    padded_shape=[P, int(slot_kb.max())])
                    denom = wpool.tile([P, 1], mybir.dt.float32, tag="dn")
                    nc.scalar.activation(ev[:], a2[:],
                                         mybir.ActivationFunctionType.Exp,
                                         accum_out=denom[:])
                    msg = wpool.tile([P, kb * C], mybir.dt.bfloat16, tag="msg",
                                     padded_shape=[P, int(slot_kb.max()) * C])
                    nc.vector.tensor_tensor(
                        out=msg[:].rearrange("p (k c) -> p k c", c=C),
                        in0=g3[:, :, 0:C],
                        in1=ev[:].to_broadcast([P, kb, C]),
                        op=mybir.AluOpType.mult)
                    h = int(kb // 2)
                    hh = h + (kb % 2)
                    m2 = wpool.tile(
                        [P, hh * C], mybir.dt.bfloat16, tag="m2",
                        padded_shape=[P, (int(slot_kb.max()) // 2 + 1) * C])
                    if h > 0:
                        nc.vector.tensor_tensor(
                            out=m2[:, 0:h * C], in0=msg[:, 0:h * C],
                            in1=msg[:, h * C:2 * h * C],
                            op=mybir.AluOpType.add)
                    if kb % 2:
                        nc.scalar.copy(out=m2[:, h * C:hh * C],
                                       in_=msg[:, 2 * h * C:kb * C])
                    num = wpool.tile([P, C], mybir.dt.float32, tag="num")
                    nc.vector.tensor_reduce(
                        out=num[:],
                        in_=m2[:].rearrange("p (k c) -> p c k", c=C),
                        axis=mybir.AxisListType.X, op=mybir.AluOpType.add)
                    dn2 = wpool.tile([P, 1], mybir.dt.float32, tag="dn2")
                    nc.vector.tensor_scalar(out=dn2[:], in0=denom[:],
                                            scalar1=1e-16, scalar2=None,
                                            op0=mybir.AluOpType.add)
                    rec = wpool.tile([P, 1], mybir.dt.float32, tag="rec")
                    nc.vector.reciprocal(rec[:], dn2[:])
                    if layer == 0:
                        nc.vector.tensor_scalar(
                            out=h0_sb[:, b * C:(b + 1) * C], in0=num[:],
                            scalar1=rec[:, 0:1], scalar2=0.0,
                            op0=mybir.AluOpType.mult, op1=mybir.AluOpType.max)
                    else:
                        h1 = wpool.tile([P, C], mybir.dt.float32, tag="h1")
                        nc.vector.tensor_scalar(
                            out=h1[:], in0=num[:],
                            scalar1=rec[:, 0:1], scalar2=0.0,
                            op0=mybir.AluOpType.mult, op1=mybir.AluOpType.max)
                        tp2 = psB.tile([C, P], mybir.dt.float32, tag="tp")
                        nc.tensor.transpose(out=tp2[:], in_=h1[:],
                                            identity=ident[:])
                        h1T = wpool.tile([C, P], mybir.dt.float32, tag="h1T")
                        nc.vector.tensor_copy(out=h1T[:], in_=tp2[:])
                        po = psA.tile([P, C], mybir.dt.float32, tag="po")
                        nc.tensor.matmul(out=po[:], lhsT=h1T[:], rhs=wp_sb[:],
                                         start=True, stop=True)
                        ot = wpool.tile([P, C], mybir.dt.float32, tag="ot")
                        nc.vector.tensor_copy(out=ot[:], in_=po[:])
                        nc.sync.dma_start(out_d[b * P:(b + 1) * P, :], ot[:])

            build_table(0)
            edge_phase(0)
            build_table(1)
            edge_phase(1)

    nc.finalize()
    return nc


def kernel(x, edge_index, W0, al0, ar0, W1, al1, ar1, Wp1, bp1, Wp2, bp2):
    x = np.asarray(x, np.float32)
    node_order, inv_node_order, slot_kb, offs, idx = _preprocess(
        np.asarray(edge_index))
    sumk = int(slot_kb.sum())

    # weight packing (host)
    al0v = np.asarray(al0, np.float32).reshape(-1)
    ar0v = np.asarray(ar0, np.float32).reshape(-1)
    al1v = np.asarray(al1, np.float32).reshape(-1)
    ar1v = np.asarray(ar1, np.float32).reshape(-1)
    W0 = np.asarray(W0, np.float32)
    W1 = np.asarray(W1, np.float32)
    w0ext = np.column_stack([W0, W0 @ al0v, W0 @ ar0v]).astype(BF16)
    w1ext = np.column_stack([W1, W1 @ al1v, W1 @ ar1v]).astype(BF16)
    Wp = (np.asarray(Wp1, np.float32) @ np.asarray(Wp2, np.float32))
    bp = (np.asarray(bp1, np.float32) @ np.asarray(Wp2, np.float32)
          + np.asarray(bp2, np.float32))

    x_perm = np.zeros((NPAD, IN_DIM), np.float32)
    x_perm[inv_node_order[np.arange(N)]] = x
    xT = np.ascontiguousarray(x_perm.T.astype(BF16))   # [128, NPAD]

    nc = _build_program(slot_kb, offs, sumk)
    in_maps = []
    for i in range(NCORES):
        in_maps.append({
            "xT": np.ascontiguousarray(
                xT[:, i * SLICE_NODES:(i + 1) * SLICE_NODES]),
            "idx_in": np.ascontiguousarray(idx[i]),
            "w0": w0ext, "w1": w1ext, "wp": Wp.astype(np.float32),
        })
    res = run_bass_kernel_spmd(nc, in_maps, core_ids=list(range(NCORES)))
    global _LAST_RESULTS
    _LAST_RESULTS = res
    out_perm = np.concatenate(
        [res.results[i]["out_d"] for i in range(NCORES)], axis=0)  # [NPAD, C]
    out = out_perm[inv_node_order[np.arange(N)]] + bp
    return out.astype(np.float32)



# revision 26
# speedup vs baseline: 1.0198x; 1.0198x over previous
"""Trainium2 Bass kernel for 2-layer GAT (N=100000, E=1600000, 64-dim) + MLP.

Strategy (dst-sharded, 8 cores):
- Host: permute nodes by in-degree into 128-node blocks; slot j across cores
  shares a compile-time max-degree Kb. Per-core gather index grids [128, Kb]
  (partition = dst node, k = in-edge slot), padded with a table pad-row whose
  attention logit is -1e9 (=> exp ~ 0).
- Device per layer: build feature table slice [xh(64)|al_hi|al_lo] bf16 via
  one matmul with host-packed extended weights (al = xh@att_l folded into the
  weight matrix; ar kept per-node on-chip). AllGather table slices across the
  8 cores. Per block: indirect-DMA gather of in-edge rows, attention weights
  via ACT (lrelu+exp, segment-max skipped - logits are O(10)), weighted sum
  via DVE mult + reduce, normalize + relu. Layer-1 table built from layer-0
  output resident in SBUF; post-MLP (two linears folded into one matmul on
  host) fused into the layer-1 epilogue.
- Host: concat per-core output slices, inverse-permute.
"""
import sys

for _p in ('/opt/trn_rl_repo', '/root/.axon_site/_ro/trn_rl_repo'):
    if _p not in sys.path:
        sys.path.insert(0, _p)

import numpy as np
import ml_dtypes

import concourse.bass as bass
import concourse.bacc as bacc
import concourse.mybir as mybir
import concourse.tile as tile
from concourse.bass_utils import run_bass_kernel_spmd
from concourse.masks import make_identity

BF16 = ml_dtypes.bfloat16
P = 128
NCORES = 8
N = 100000
E = 1600000
IN_DIM = 128
C = 64           # hidden dim
CT = 66          # table cols: 64 features + al_hi + al_lo
NEG_SLOPE = 0.2

NPAD = ((N + NCORES * P - 1) // (NCORES * P)) * (NCORES * P)   # 100352
N_BLOCKS = NPAD // P                                           # 784
N_SLOTS = N_BLOCKS // NCORES                                   # 98
SLICE_NODES = N_SLOTS * P                                      # 12544
SLICE_H = SLICE_NODES + 1                                      # + pad row
PAD_ROW = SLICE_NODES                                          # global table row
TBL_H = SLICE_H * NCORES


def _preprocess(edge_index):
    src = edge_index[0].astype(np.int64)
    dst = edge_index[1].astype(np.int64)
    deg = np.bincount(dst, minlength=N)
    deg_pad = np.concatenate([deg, np.zeros(NPAD - N, np.int64)])
    perm = np.argsort(-deg_pad, kind="stable")          # perm[new_pos] = node
    # block g (sorted order) -> max degree
    blk_max = deg_pad[perm].reshape(N_BLOCKS, P).max(axis=1)
    slot_kb = np.maximum(
        blk_max.reshape(N_SLOTS, NCORES).max(axis=1), 1).astype(np.int64)
    # core i gets sorted blocks (8j + i); node order = core-major
    block_order = np.concatenate([np.arange(N_SLOTS) * NCORES + i for i in range(NCORES)])
    node_order = perm.reshape(N_BLOCKS, P)[block_order].reshape(-1)
    inv_node_order = np.empty(NPAD, np.int64)
    inv_node_order[node_order] = np.arange(NPAD)

    # CSR of in-edges per node
    order = np.argsort(dst, kind="stable")
    src_sorted = src[order]
    starts = np.zeros(N + 1, np.int64)
    np.cumsum(deg, out=starts[1:])

    # table row of node m (in AG layout)
    q = inv_node_order  # permuted position of node m
    tbl_row = (q // SLICE_NODES) * SLICE_H + (q % SLICE_NODES)   # [NPAD]

    sumk = int(slot_kb.sum())
    idx = np.full((NCORES, P, sumk), PAD_ROW, np.int32)
    offs = np.zeros(N_SLOTS + 1, np.int64)
    np.cumsum(slot_kb, out=offs[1:])
    for i in range(NCORES):
        for j in range(N_SLOTS):
            o = offs[j]
            for p in range(P):
                m = node_order[i * SLICE_NODES + j * P + p]
                if m < N:
                    s, e = starts[m], starts[m + 1]
                    rows = tbl_row[src_sorted[s:e]]
                    idx[i, p, o:o + (e - s)] = rows
    return node_order, inv_node_order, slot_kb, offs, idx


def _build_program(slot_kb, offs, sumk):
    nc = bacc.Bacc("TRN2", target_bir_lowering=False, debug=False,
                   num_devices=NCORES)
    xT = nc.dram_tensor("xT", [IN_DIM, SLICE_NODES], mybir.dt.bfloat16,
                        kind="ExternalInput")
    idx_in = nc.dram_tensor("idx_in", [P, sumk], mybir.dt.int32,
                            kind="ExternalInput")
    w0 = nc.dram_tensor("w0", [IN_DIM, CT], mybir.dt.bfloat16, kind="ExternalInput")
    w1 = nc.dram_tensor("w1", [C, CT], mybir.dt.bfloat16, kind="ExternalInput")
    wp = nc.dram_tensor("wp", [C, C], mybir.dt.float32, kind="ExternalInput")
    out_d = nc.dram_tensor("out_d", [SLICE_NODES, C], mybir.dt.float32,
                           kind="ExternalOutput")

    with tile.TileContext(nc) as tc:
        with (
            tc.tile_pool(name="dram", bufs=1, space="DRAM") as dram,
            tc.tile_pool(name="const", bufs=1) as cpool,
            tc.tile_pool(name="persist", bufs=1) as ppool,
            tc.tile_pool(name="work", bufs=6) as wpool,
            tc.tile_pool(name="psA", bufs=2, space="PSUM") as psA,
            tc.tile_pool(name="psB", bufs=2, space="PSUM") as psB,
        ):
            ident = cpool.tile([P, P], mybir.dt.float32)
            make_identity(nc, ident)
            w0_sb = cpool.tile([IN_DIM, CT], mybir.dt.bfloat16)
            nc.sync.dma_start(w0_sb[:], w0[:])
            w1_sb = cpool.tile([C, CT], mybir.dt.bfloat16)
            nc.sync.dma_start(w1_sb[:], w1[:])
            wp_sb = cpool.tile([C, C], mybir.dt.float32)
            nc.sync.dma_start(wp_sb[:], wp[:])
            idx_sb = ppool.tile([P, sumk], mybir.dt.int32)
            nc.sync.dma_start(idx_sb[:], idx_in[:])

            # bulk-load x^T once (4 large DMAs instead of 98 small ones)
            xT_sb = ppool.tile([IN_DIM, SLICE_NODES], mybir.dt.bfloat16)
            QTR = SLICE_NODES // 4
            for q in range(4):
                nc.sync.dma_start(xT_sb[:, q * QTR:(q + 1) * QTR],
                                  xT[:, q * QTR:(q + 1) * QTR])

            h0_sb = ppool.tile([P, N_SLOTS * C], mybir.dt.float32)
            ar_sb = ppool.tile([P, 2 * N_SLOTS], mybir.dt.float32)

            tabs = []
            for layer in range(2):
                tab_slice = dram.tile([SLICE_H, CT], mybir.dt.bfloat16,
                                      name=f"tab_slice{layer}")
                tab_full = dram.tile([TBL_H, CT], mybir.dt.bfloat16,
                                     addr_space="Shared", name=f"tab_full{layer}")
                tabs.append((tab_slice, tab_full))

            padrow = cpool.tile([1, CT], mybir.dt.bfloat16)
            nc.vector.memset(padrow[:], 0)
            nc.vector.memset(padrow[0:1, C:C + 1], -1e9)

            def build_table(layer):
                tab_slice, tab_full = tabs[layer]
                for b in range(N_SLOTS):
                    ps = psA.tile([P, CT], mybir.dt.float32, tag="pst")
                    if layer == 0:
                        nc.tensor.matmul(out=ps[:],
                                         lhsT=xT_sb[:, b * P:(b + 1) * P],
                                         rhs=w0_sb[:], start=True, stop=True)
                    else:
                        tp = psB.tile([C, P], mybir.dt.float32, tag="tp")
                        nc.tensor.transpose(
                            out=tp[:], in_=h0_sb[:, b * C:(b + 1) * C],
                            identity=ident[:])
                        lhs = wpool.tile([C, P], mybir.dt.bfloat16, tag="hT")
                        nc.vector.tensor_copy(out=lhs[:], in_=tp[:])
                        nc.tensor.matmul(out=ps[:], lhsT=lhs[:], rhs=w1_sb[:],
                                         start=True, stop=True)
                    tt = wpool.tile([P, CT], mybir.dt.bfloat16, tag="tt")
                    nc.vector.tensor_copy(out=tt[:, 0:C + 1], in_=ps[:, 0:C + 1])
                    # al_lo = al - bf16(al)
                    nc.vector.tensor_tensor(
                        out=tt[:, C + 1:C + 2], in0=ps[:, C:C + 1],
                        in1=tt[:, C:C + 1], op=mybir.AluOpType.subtract)
                    # keep ar
                    nc.scalar.copy(
                        out=ar_sb[:, layer * N_SLOTS + b:layer * N_SLOTS + b + 1],
                        in_=ps[:, C + 1:C + 2])
                    nc.sync.dma_start(tab_slice[b * P:(b + 1) * P, :], tt[:])
                nc.sync.dma_start(tab_slice[PAD_ROW:PAD_ROW + 1, :], padrow[:])
                nc.gpsimd.collective_compute(
                    "AllGather", mybir.AluOpType.bypass,
                    replica_groups=[list(range(NCORES))],
                    ins=[tab_slice[:]], outs=[tab_full[:]],
                )

            def edge_phase(layer):
                _, tab_full = tabs[layer]
                for b in range(N_SLOTS):
                    kb = int(slot_kb[b])
                    o = int(offs[b])
                    g = wpool.tile([P, kb * CT], mybir.dt.bfloat16, tag="g",
                                   padded_shape=[P, int(slot_kb.max()) * CT])
                    for k in range(kb):
                        nc.gpsimd.indirect_dma_start(
                            out=g[:, k * CT:(k + 1) * CT],
                            out_offset=None,
                            in_=tab_full[:],
                            in_offset=bass.IndirectOffsetOnAxis(
                                ap=idx_sb[:, o + k:o + k + 1], axis=0),
                        )
                    g3 = g[:].rearrange("p (k c) -> p k c", c=CT)
                    a_t = wpool.tile([P, kb], mybir.dt.float32, tag="a",
                                     padded_shape=[P, int(slot_kb.max())])
                    nc.vector.tensor_tensor(out=a_t[:], in0=g3[:, :, C],
                                            in1=g3[:, :, C + 1],
                                            op=mybir.AluOpType.add)
                    af = wpool.tile([P, kb], mybir.dt.float32, tag="af",
                                    padded_shape=[P, int(slot_kb.max())])
                    nc.vector.tensor_scalar(
                        out=af[:], in0=a_t[:],
                        scalar1=ar_sb[:, layer * N_SLOTS + b:layer * N_SLOTS + b + 1],
                        scalar2=None, op0=mybir.AluOpType.add)
                    am = wpool.tile([P, kb], mybir.dt.float32, tag="am",
                                    padded_shape=[P, int(slot_kb.max())])
                    nc.vector.tensor_scalar(
                        out=am[:], in0=af[:], scalar1=NEG_SLOPE, scalar2=None,
                        op0=mybir.AluOpType.mult)
                    a2 = wpool.tile([P, kb], mybir.dt.float32, tag="a2",
                                    padded_shape=[P, int(slot_kb.max())])
                    nc.vector.tensor_tensor(out=a2[:], in0=af[:], in1=am[:],
                                            op=mybir.AluOpType.max)
                    ev = wpool.tile([P, kb], mybir.dt.bfloat16, tag="ev",
                                    padded_shape=[P, int(slot_kb.max())])
                    denom = wpool.tile([P, 1], mybir.dt.float32, tag="dn")
                    nc.scalar.activation(ev[:], a2[:],
                                         mybir.ActivationFunctionType.Exp,
                                         accum_out=denom[:])
                    msg = wpool.tile([P, kb * C], mybir.dt.bfloat16, tag="msg",
                                     padded_shape=[P, int(slot_kb.max()) * C])
                    nc.vector.tensor_tensor(
                        out=msg[:].rearrange("p (k c) -> p k c", c=C),
                        in0=g3[:, :, 0:C],
                        in1=ev[:].to_broadcast([P, kb, C]),
                        op=mybir.AluOpType.mult)
                    h = int(kb // 2)
                    hh = h + (kb % 2)
                    m2 = wpool.tile(
                        [P, hh * C], mybir.dt.bfloat16, tag="m2",
                        padded_shape=[P, (int(slot_kb.max()) // 2 + 1) * C])
                    if h > 0:
                        nc.vector.tensor_tensor(
                            out=m2[:, 0:h * C], in0=msg[:, 0:h * C],
                            in1=msg[:, h * C:2 * h * C],
                            op=mybir.AluOpType.add)
                    if kb % 2:
                        nc.scalar.copy(out=m2[:, h * C:hh * C],
                                       in_=msg[:, 2 * h * C:kb * C])
                    num = wpool.tile([P, C], mybir.dt.float32, tag="num")
                    nc.vector.tensor_reduce(
                        out=num[:],
                        in_=m2[:].rearrange("p (k c) -> p c k", c=C),
                        axis=mybir.AxisListType.X, op=mybir.AluOpType.add)
                    dn2 = wpool.tile([P, 1], mybir.dt.float32, tag="dn2")
                    nc.vector.tensor_scalar(out=dn2[:], in0=denom[:],
                                            scalar1=1e-16, scalar2=None,
                                            op0=mybir.AluOpType.add)
                    rec = wpool.tile([P, 1], mybir.dt.float32, tag="rec")
                    nc.vector.reciprocal(rec[:], dn2[:])
                    if layer == 0:
                        nc.vector.tensor_scalar(
                            out=h0_sb[:, b * C:(b + 1) * C], in0=num[:],
                            scalar1=rec[:, 0:1], scalar2=0.0,
                            op0=mybir.AluOpType.mult, op1=mybir.AluOpType.max)
                    else:
                        h1 = wpool.tile([P, C], mybir.dt.float32, tag="h1")
                        nc.vector.tensor_scalar(
                            out=h1[:], in0=num[:],
                            scalar1=rec[:, 0:1], scalar2=0.0,
                            op0=mybir.AluOpType.mult, op1=mybir.AluOpType.max)
                        tp2 = psB.tile([C, P], mybir.dt.float32, tag="tp")
                        nc.tensor.transpose(out=tp2[:], in_=h1[:],
                                            identity=ident[:])
                        h1T = wpool.tile([C, P], mybir.dt.float32, tag="h1T")
                        nc.vector.tensor_copy(out=h1T[:], in_=tp2[:])
                        po = psA.tile([P, C], mybir.dt.float32, tag="po")
                        nc.tensor.matmul(out=po[:], lhsT=h1T[:], rhs=wp_sb[:],
                                         start=True, stop=True)
                        ot = wpool.tile([P, C], mybir.dt.float32, tag="ot")
                        nc.vector.tensor_copy(out=ot[:], in_=po[:])
                        nc.sync.dma_start(out_d[b * P:(b + 1) * P, :], ot[:])

            build_table(0)
            edge_phase(0)
            build_table(1)
            edge_phase(1)

    nc.finalize()
    return nc


def kernel(x, edge_index, W0, al0, ar0, W1, al1, ar1, Wp1, bp1, Wp2, bp2):
    x = np.asarray(x, np.float32)
    node_order, inv_node_order, slot_kb, offs, idx = _preprocess(
        np.asarray(edge_index))
    sumk = int(slot_kb.sum())

    # weight packing (host)
    al0v = np.asarray(al0, np.float32).reshape(-1)
    ar0v = np.asarray(ar0, np.float32).reshape(-1)
    al1v = np.asarray(al1, np.float32).reshape(-1)
    ar1v = np.asarray(ar1, np.float32).reshape(-1)
    W0 = np.asarray(W0, np.float32)
    W1 = np.asarray(W1, np.float32)
    w0ext = np.column_stack([W0, W0 @ al0v, W0 @ ar0v]).astype(BF16)
    w1ext = np.column_stack([W1, W1 @ al1v, W1 @ ar1v]).astype(BF16)
    Wp = (np.asarray(Wp1, np.float32) @ np.asarray(Wp2, np.float32))
    bp = (np.asarray(bp1, np.float32) @ np.asarray(Wp2, np.float32)
          + np.asarray(bp2, np.float32))

    x_perm = np.zeros((NPAD, IN_DIM), np.float32)
    x_perm[inv_node_order[np.arange(N)]] = x
    xT = np.ascontiguousarray(x_perm.T.astype(BF16))   # [128, NPAD]

    nc = _build_program(slot_kb, offs, sumk)
    in_maps = []
    for i in range(NCORES):
        in_maps.append({
            "xT": np.ascontiguousarray(
                xT[:, i * SLICE_NODES:(i + 1) * SLICE_NODES]),
            "idx_in": np.ascontiguousarray(idx[i]),
            "w0": w0ext, "w1": w1ext, "wp": Wp.astype(np.float32),
        })
    res = run_bass_kernel_spmd(nc, in_maps, core_ids=list(range(NCORES)))
    global _LAST_RESULTS
    _LAST_RESULTS = res
    out_perm = np.concatenate(
        [res.results[i]["out_d"] for i in range(NCORES)], axis=0)  # [NPAD, C]
    out = out_perm[inv_node_order[np.arange(N)]] + bp
    return out.astype(np.float32)



# revision 27
# speedup vs baseline: 1.0235x; 1.0036x over previous
"""Trainium2 Bass kernel for 2-layer GAT (N=100000, E=1600000, 64-dim) + MLP.

Strategy (dst-sharded, 8 cores):
- Host: permute nodes by in-degree into 128-node blocks; slot j across cores
  shares a compile-time max-degree Kb. Per-core gather index grids [128, Kb]
  (partition = dst node, k = in-edge slot), padded with a table pad-row whose
  attention logit is -1e9 (=> exp ~ 0).
- Device per layer: build feature table slice [xh(64)|al_hi|al_lo] bf16 via
  one matmul with host-packed extended weights (al = xh@att_l folded into the
  weight matrix; ar kept per-node on-chip). AllGather table slices across the
  8 cores. Per block: indirect-DMA gather of in-edge rows, attention weights
  via ACT (lrelu+exp, segment-max skipped - logits are O(10)), weighted sum
  via DVE mult + reduce, normalize + relu. Layer-1 table built from layer-0
  output resident in SBUF; post-MLP (two linears folded into one matmul on
  host) fused into the layer-1 epilogue.
- Host: concat per-core output slices, inverse-permute.
"""
import sys

for _p in ('/opt/trn_rl_repo', '/root/.axon_site/_ro/trn_rl_repo'):
    if _p not in sys.path:
        sys.path.insert(0, _p)

import numpy as np
import ml_dtypes

import concourse.bass as bass
import concourse.bacc as bacc
import concourse.mybir as mybir
import concourse.tile as tile
from concourse.bass_utils import run_bass_kernel_spmd
from concourse.masks import make_identity

BF16 = ml_dtypes.bfloat16
P = 128
NCORES = 8
N = 100000
E = 1600000
IN_DIM = 128
C = 64           # hidden dim
CT = 66          # table cols: 64 features + al_hi + al_lo
NEG_SLOPE = 0.2

NPAD = ((N + NCORES * P - 1) // (NCORES * P)) * (NCORES * P)   # 100352
N_BLOCKS = NPAD // P                                           # 784
N_SLOTS = N_BLOCKS // NCORES                                   # 98
SLICE_NODES = N_SLOTS * P                                      # 12544
SLICE_H = SLICE_NODES + 1                                      # + pad row
PAD_ROW = SLICE_NODES                                          # global table row
TBL_H = SLICE_H * NCORES


def _preprocess(edge_index):
    src = edge_index[0].astype(np.int64)
    dst = edge_index[1].astype(np.int64)
    deg = np.bincount(dst, minlength=N)
    deg_pad = np.concatenate([deg, np.zeros(NPAD - N, np.int64)])
    perm = np.argsort(-deg_pad, kind="stable")          # perm[new_pos] = node
    # block g (sorted order) -> max degree
    blk_max = deg_pad[perm].reshape(N_BLOCKS, P).max(axis=1)
    slot_kb = np.maximum(
        blk_max.reshape(N_SLOTS, NCORES).max(axis=1), 1).astype(np.int64)
    # core i gets sorted blocks (8j + i); node order = core-major
    block_order = np.concatenate([np.arange(N_SLOTS) * NCORES + i for i in range(NCORES)])
    node_order = perm.reshape(N_BLOCKS, P)[block_order].reshape(-1)
    inv_node_order = np.empty(NPAD, np.int64)
    inv_node_order[node_order] = np.arange(NPAD)

    # CSR of in-edges per node
    order = np.argsort(dst, kind="stable")
    src_sorted = src[order]
    starts = np.zeros(N + 1, np.int64)
    np.cumsum(deg, out=starts[1:])

    # table row of node m (in AG layout)
    q = inv_node_order  # permuted position of node m
    tbl_row = (q // SLICE_NODES) * SLICE_H + (q % SLICE_NODES)   # [NPAD]

    sumk = int(slot_kb.sum())
    idx = np.full((NCORES, P, sumk), PAD_ROW, np.int32)
    offs = np.zeros(N_SLOTS + 1, np.int64)
    np.cumsum(slot_kb, out=offs[1:])
    for i in range(NCORES):
        for j in range(N_SLOTS):
            o = offs[j]
            for p in range(P):
                m = node_order[i * SLICE_NODES + j * P + p]
                if m < N:
                    s, e = starts[m], starts[m + 1]
                    rows = tbl_row[src_sorted[s:e]]
                    idx[i, p, o:o + (e - s)] = rows
    return node_order, inv_node_order, slot_kb, offs, idx


def _build_program(slot_kb, offs, sumk):
    nc = bacc.Bacc("TRN2", target_bir_lowering=False, debug=False,
                   num_devices=NCORES)
    xT = nc.dram_tensor("xT", [IN_DIM, SLICE_NODES], mybir.dt.bfloat16,
                        kind="ExternalInput")
    idx_in = nc.dram_tensor("idx_in", [P, sumk], mybir.dt.int32,
                            kind="ExternalInput")
    w0 = nc.dram_tensor("w0", [IN_DIM, CT], mybir.dt.bfloat16, kind="ExternalInput")
    w1 = nc.dram_tensor("w1", [C, CT], mybir.dt.bfloat16, kind="ExternalInput")
    wp = nc.dram_tensor("wp", [C, C], mybir.dt.float32, kind="ExternalInput")
    out_d = nc.dram_tensor("out_d", [SLICE_NODES, C], mybir.dt.float32,
                           kind="ExternalOutput")

    with tile.TileContext(nc) as tc:
        with (
            tc.tile_pool(name="dram", bufs=1, space="DRAM") as dram,
            tc.tile_pool(name="const", bufs=1) as cpool,
            tc.tile_pool(name="persist", bufs=1) as ppool,
            tc.tile_pool(name="work", bufs=6) as wpool,
            tc.tile_pool(name="psA", bufs=2, space="PSUM") as psA,
            tc.tile_pool(name="psB", bufs=2, space="PSUM") as psB,
        ):
            ident = cpool.tile([P, P], mybir.dt.float32)
            make_identity(nc, ident)
            w0_sb = cpool.tile([IN_DIM, CT], mybir.dt.bfloat16)
            nc.sync.dma_start(w0_sb[:], w0[:])
            w1_sb = cpool.tile([C, CT], mybir.dt.bfloat16)
            nc.sync.dma_start(w1_sb[:], w1[:])
            wp_sb = cpool.tile([C, C], mybir.dt.float32)
            nc.sync.dma_start(wp_sb[:], wp[:])
            idx_sb = ppool.tile([P, sumk], mybir.dt.int32)
            nc.sync.dma_start(idx_sb[:], idx_in[:])

            # bulk-load x^T once (4 large DMAs instead of 98 small ones)
            xT_sb = ppool.tile([IN_DIM, SLICE_NODES], mybir.dt.bfloat16)
            QTR = SLICE_NODES // 4
            for q in range(4):
                nc.sync.dma_start(xT_sb[:, q * QTR:(q + 1) * QTR],
                                  xT[:, q * QTR:(q + 1) * QTR])

            h0_sb = ppool.tile([P, N_SLOTS * C], mybir.dt.float32)
            ar_sb = ppool.tile([P, 2 * N_SLOTS], mybir.dt.float32)

            tabs = []
            for layer in range(2):
                tab_slice = dram.tile([SLICE_H, CT], mybir.dt.bfloat16,
                                      name=f"tab_slice{layer}")
                tab_full = dram.tile([TBL_H, CT], mybir.dt.bfloat16,
                                     addr_space="Shared", name=f"tab_full{layer}")
                tabs.append((tab_slice, tab_full))

            padrow = cpool.tile([1, CT], mybir.dt.bfloat16)
            nc.vector.memset(padrow[:], 0)
            nc.vector.memset(padrow[0:1, C:C + 1], -1e9)

            GRP = 4   # table blocks per write DMA

            def build_table(layer):
                tab_slice, tab_full = tabs[layer]
                tt4 = None
                for b in range(N_SLOTS):
                    ps = psA.tile([P, CT], mybir.dt.float32, tag="pst")
                    if layer == 0:
                        nc.tensor.matmul(out=ps[:],
                                         lhsT=xT_sb[:, b * P:(b + 1) * P],
                                         rhs=w0_sb[:], start=True, stop=True)
                    else:
                        tp = psB.tile([C, P], mybir.dt.float32, tag="tp")
                        nc.tensor.transpose(
                            out=tp[:], in_=h0_sb[:, b * C:(b + 1) * C],
                            identity=ident[:])
                        lhs = wpool.tile([C, P], mybir.dt.bfloat16, tag="hT")
                        nc.vector.tensor_copy(out=lhs[:], in_=tp[:])
                        nc.tensor.matmul(out=ps[:], lhsT=lhs[:], rhs=w1_sb[:],
                                         start=True, stop=True)
                    if b % GRP == 0:
                        b0 = b
                        ng = min(GRP, N_SLOTS - b0)
                        tt4 = wpool.tile([P, ng * CT], mybir.dt.bfloat16,
                                         tag="tt4", padded_shape=[P, GRP * CT])
                    c = b - b0
                    tt = tt4[:, c * CT:(c + 1) * CT]
                    nc.vector.tensor_copy(out=tt[:, 0:C + 1], in_=ps[:, 0:C + 1])
                    # al_lo = al - bf16(al)
                    nc.vector.tensor_tensor(
                        out=tt[:, C + 1:C + 2], in0=ps[:, C:C + 1],
                        in1=tt[:, C:C + 1], op=mybir.AluOpType.subtract)
                    # keep ar
                    nc.scalar.copy(
                        out=ar_sb[:, layer * N_SLOTS + b:layer * N_SLOTS + b + 1],
                        in_=ps[:, C + 1:C + 2])
                    if c == ng - 1:
                        # one DMA for ng row-blocks: DRAM row c*P+p comes from
                        # partition p, chunk c of tt4
                        nc.sync.dma_start(
                            tab_slice[b0 * P:(b0 + ng) * P, :].rearrange(
                                "(c p) f -> p c f", p=P),
                            tt4[:].rearrange("p (c f) -> p c f", f=CT))
                nc.sync.dma_start(tab_slice[PAD_ROW:PAD_ROW + 1, :], padrow[:])
                nc.gpsimd.collective_compute(
                    "AllGather", mybir.AluOpType.bypass,
                    replica_groups=[list(range(NCORES))],
                    ins=[tab_slice[:]], outs=[tab_full[:]],
                )

            def edge_phase(layer):
                _, tab_full = tabs[layer]
                for b in range(N_SLOTS):
                    kb = int(slot_kb[b])
                    o = int(offs[b])
                    g = wpool.tile([P, kb * CT], mybir.dt.bfloat16, tag="g",
                                   padded_shape=[P, int(slot_kb.max()) * CT])
                    for k in range(kb):
                        nc.gpsimd.indirect_dma_start(
                            out=g[:, k * CT:(k + 1) * CT],
                            out_offset=None,
                            in_=tab_full[:],
                            in_offset=bass.IndirectOffsetOnAxis(
                                ap=idx_sb[:, o + k:o + k + 1], axis=0),
                        )
                    g3 = g[:].rearrange("p (k c) -> p k c", c=CT)
                    a_t = wpool.tile([P, kb], mybir.dt.float32, tag="a",
                                     padded_shape=[P, int(slot_kb.max())])
                    nc.vector.tensor_tensor(out=a_t[:], in0=g3[:, :, C],
                                            in1=g3[:, :, C + 1],
                                            op=mybir.AluOpType.add)
                    af = wpool.tile([P, kb], mybir.dt.float32, tag="af",
                                    padded_shape=[P, int(slot_kb.max())])
                    nc.vector.tensor_scalar(
                        out=af[:], in0=a_t[:],
                        scalar1=ar_sb[:, layer * N_SLOTS + b:layer * N_SLOTS + b + 1],
                        scalar2=None, op0=mybir.AluOpType.add)
                    am = wpool.tile([P, kb], mybir.dt.float32, tag="am",
                                    padded_shape=[P, int(slot_kb.max())])
                    nc.vector.tensor_scalar(
                        out=am[:], in0=af[:], scalar1=NEG_SLOPE, scalar2=None,
                        op0=mybir.AluOpType.mult)
                    a2 = wpool.tile([P, kb], mybir.dt.float32, tag="a2",
                                    padded_shape=[P, int(slot_kb.max())])
                    nc.vector.tensor_tensor(out=a2[:], in0=af[:], in1=am[:],
                                            op=mybir.AluOpType.max)
                    ev = wpool.tile([P, kb], mybir.dt.bfloat16, tag="ev",
                                    padded_shape=[P, int(slot_kb.max())])
                    denom = wpool.tile([P, 1], mybir.dt.float32, tag="dn")
                    nc.scalar.activation(ev[:], a2[:],
                                         mybir.ActivationFunctionType.Exp,
                                         accum_out=denom[:])
                    msg = wpool.tile([P, kb * C], mybir.dt.bfloat16, tag="msg",
                                     padded_shape=[P, int(slot_kb.max()) * C])
                    nc.vector.tensor_tensor(
                        out=msg[:].rearrange("p (k c) -> p k c", c=C),
                        in0=g3[:, :, 0:C],
                        in1=ev[:].to_broadcast([P, kb, C]),
                        op=mybir.AluOpType.mult)
                    h = int(kb // 2)
                    hh = h + (kb % 2)
                    m2 = wpool.tile(
                        [P, hh * C], mybir.dt.bfloat16, tag="m2",
                        padded_shape=[P, (int(slot_kb.max()) // 2 + 1) * C])
                    if h > 0:
                        nc.vector.tensor_tensor(
                            out=m2[:, 0:h * C], in0=msg[:, 0:h * C],
                            in1=msg[:, h * C:2 * h * C],
                            op=mybir.AluOpType.add)
                    if kb % 2:
                        nc.scalar.copy(out=m2[:, h * C:hh * C],
                                       in_=msg[:, 2 * h * C:kb * C])
                    num = wpool.tile([P, C], mybir.dt.float32, tag="num")
                    nc.vector.tensor_reduce(
                        out=num[:],
                        in_=m2[:].rearrange("p (k c) -> p c k", c=C),
                        axis=mybir.AxisListType.X, op=mybir.AluOpType.add)
                    dn2 = wpool.tile([P, 1], mybir.dt.float32, tag="dn2")
                    nc.vector.tensor_scalar(out=dn2[:], in0=denom[:],
                                            scalar1=1e-16, scalar2=None,
                                            op0=mybir.AluOpType.add)
                    rec = wpool.tile([P, 1], mybir.dt.float32, tag="rec")
                    nc.vector.reciprocal(rec[:], dn2[:])
                    if layer == 0:
                        nc.vector.tensor_scalar(
                            out=h0_sb[:, b * C:(b + 1) * C], in0=num[:],
                            scalar1=rec[:, 0:1], scalar2=0.0,
                            op0=mybir.AluOpType.mult, op1=mybir.AluOpType.max)
                    else:
                        h1 = wpool.tile([P, C], mybir.dt.float32, tag="h1")
                        nc.vector.tensor_scalar(
                            out=h1[:], in0=num[:],
                            scalar1=rec[:, 0:1], scalar2=0.0,
                            op0=mybir.AluOpType.mult, op1=mybir.AluOpType.max)
                        tp2 = psB.tile([C, P], mybir.dt.float32, tag="tp")
                        nc.tensor.transpose(out=tp2[:], in_=h1[:],
                                            identity=ident[:])
                        h1T = wpool.tile([C, P], mybir.dt.float32, tag="h1T")
                        nc.vector.tensor_copy(out=h1T[:], in_=tp2[:])
                        po = psA.tile([P, C], mybir.dt.float32, tag="po")
                        nc.tensor.matmul(out=po[:], lhsT=h1T[:], rhs=wp_sb[:],
                                         start=True, stop=True)
                        ot = wpool.tile([P, C], mybir.dt.float32, tag="ot")
                        nc.vector.tensor_copy(out=ot[:], in_=po[:])
                        nc.sync.dma_start(out_d[b * P:(b + 1) * P, :], ot[:])

            build_table(0)
            edge_phase(0)
            build_table(1)
            edge_phase(1)

    nc.finalize()
    return nc


def kernel(x, edge_index, W0, al0, ar0, W1, al1, ar1, Wp1, bp1, Wp2, bp2):
    x = np.asarray(x, np.float32)
    node_order, inv_node_order, slot_kb, offs, idx = _preprocess(
        np.asarray(edge_index))
    sumk = int(slot_kb.sum())

    # weight packing (host)
    al0v = np.asarray(al0, np.float32).reshape(-1)
    ar0v = np.asarray(ar0, np.float32).reshape(-1)
    al1v = np.asarray(al1, np.float32).reshape(-1)
    ar1v = np.asarray(ar1, np.float32).reshape(-1)
    W0 = np.asarray(W0, np.float32)
    W1 = np.asarray(W1, np.float32)
    w0ext = np.column_stack([W0, W0 @ al0v, W0 @ ar0v]).astype(BF16)
    w1ext = np.column_stack([W1, W1 @ al1v, W1 @ ar1v]).astype(BF16)
    Wp = (np.asarray(Wp1, np.float32) @ np.asarray(Wp2, np.float32))
    bp = (np.asarray(bp1, np.float32) @ np.asarray(Wp2, np.float32)
          + np.asarray(bp2, np.float32))

    x_perm = np.zeros((NPAD, IN_DIM), np.float32)
    x_perm[inv_node_order[np.arange(N)]] = x
    xT = np.ascontiguousarray(x_perm.T.astype(BF16))   # [128, NPAD]

    nc = _build_program(slot_kb, offs, sumk)
    in_maps = []
    for i in range(NCORES):
        in_maps.append({
            "xT": np.ascontiguousarray(
                xT[:, i * SLICE_NODES:(i + 1) * SLICE_NODES]),
            "idx_in": np.ascontiguousarray(idx[i]),
            "w0": w0ext, "w1": w1ext, "wp": Wp.astype(np.float32),
        })
    res = run_bass_kernel_spmd(nc, in_maps, core_ids=list(range(NCORES)))
    global _LAST_RESULTS
    _LAST_RESULTS = res
    out_perm = np.concatenate(
        [res.results[i]["out_d"] for i in range(NCORES)], axis=0)  # [NPAD, C]
    out = out_perm[inv_node_order[np.arange(N)]] + bp
    return out.astype(np.float32)

